# revision 32
# baseline (speedup 1.0000x reference)
"""CPR router kernel for Trainium2 (8 NeuronCores, data-parallel over tokens).

Math (matches the jax reference):
    h_n = l2norm(hidden_states, axis=1); p_n = l2norm(proto, axis=1)
    logits = h_n @ p_n.T                      # [T, 64] cosine sims
    w = softmax(logits, axis=1)
    routing_weights, selected_experts = top_k(w, 8)

Device strategy (per core, 2048 tokens, 16 tiles of 128 tokens):
    - Host ships h PRE-TRANSPOSED per core: ht[d, t] = h[t, d], so the
      contraction dim d lands on SBUF partitions with no PE transposes and
      no PSUM->SBUF staging copies at all (those were 27us PE + 45us
      DVE/ACT of engine time in the token-major design).
    - proto is tiny: normalized + transposed on host, replicated.
    - Per 128-token tile, one DMA gathers [128p, 16 d-chunks, 128 tok]
      (512B runs -> full DMA efficiency). Then:
        PE: 16 f32 matmuls accumulate logits[128t, 64e] in PSUM
            (lhsT = ht chunk [128d, 128t], rhs = proto chunk [128d, 64e]).
        ACT or DVE (split for balance): sq = ht*ht elementwise.
        PE: 16 matmuls vs a ones[128,1] column reduce sq over partitions
            into ssq[128t, 1] PSUM at ap_size=1 (~free in PE time).
        DVE: rsqrt(ssq) via Quake seed + 2 Newton steps (rel err 4e-6,
            far under the top-k gap scale; no ACT table switch).
        ACT: Exp reads logits straight from PSUM with per-partition
            scale=inv_norm -> softmax numerator.
        DVE: reduce_sum denominator, top-8 max/max_index on the
            (unnormalized) probs -- softmax normalization is a positive
            per-token scalar so the selection is identical -- then scale
            just the top-8 values by 1/den.
    - Outputs stage in ONE u32 buffer [128, 2, 16*8] (w bitcast | idx) so
      each flush is a single DMA; host re-permutes and splits.
    Every engine sits well under the ~49us DMA floor (16.8MB of f32 h at
    358GB/s/core), so the kernel is memory-bound per the target regime.
"""

from contextlib import ExitStack

import numpy as np

import concourse.bass as bass
import concourse.bacc as bacc
import concourse.mybir as mybir
import concourse.tile as tile

N_CORES = 8
T_FULL = 16384
D = 2048
E = 64
K = 8
P = 128
T_CORE = T_FULL // N_CORES  # 2048
N_TILES = T_CORE // P       # 16
N_CHUNKS = D // P           # 16

F32 = mybir.dt.float32
U32 = mybir.dt.uint32

# Engine for each tile's sq = ht*ht pass: "a"=ACT, "v"=DVE, "g"=GPSIMD/Pool.
# ACT also runs the Exps; DVE runs rsqrt + softmax/top-k; Pool is idle.
SQ_PLAN = "av" * 8
# Newton steps after the Quake rsqrt seed (2 -> rel err ~4e-6).
NEWTON = 2
# Pipeline lag (tiles) between phase_a (DMA/square/logits) and phase_s
# (ssq matmuls + rsqrt) / phase_b (softmax/top-k).
LAG = 1


INV_MODE = "quake"


def build_program(sq_plan=None, lag=None, newton=None, inv_mode=None, reps=1):
    global SQ_PLAN, LAG, NEWTON, INV_MODE
    if sq_plan is not None:
        SQ_PLAN = sq_plan
    if lag is not None:
        LAG = lag
    if newton is not None:
        NEWTON = newton
    if inv_mode is not None:
        INV_MODE = inv_mode
    nc = bacc.Bacc(
        "TRN2", target_bir_lowering=False, debug=False, num_devices=N_CORES
    )
    ht_d = nc.dram_tensor("ht", [D, T_CORE], F32, kind="ExternalInput").ap()
    pt_d = nc.dram_tensor("pt", [P, N_CHUNKS * E], F32, kind="ExternalInput").ap()
    oc_d = nc.dram_tensor(
        "out_c", [P, 2, N_TILES * K], U32, kind="ExternalOutput"
    ).ap()

    with tile.TileContext(nc) as tc, ExitStack() as ctx:
        singles = ctx.enter_context(tc.tile_pool(name="singles", bufs=1))
        h_pool = ctx.enter_context(tc.tile_pool(name="hin", bufs=5))
        sq_pool = ctx.enter_context(tc.tile_pool(name="sq", bufs=4))
        small = ctx.enter_context(tc.tile_pool(name="small", bufs=4))
        psL_pool = ctx.enter_context(
            tc.tile_pool(name="psL", bufs=6, space=bass.MemorySpace.PSUM)
        )
        ps_single = ctx.enter_context(
            tc.tile_pool(name="pss", bufs=1, space=bass.MemorySpace.PSUM)
        )

        pt_sb = singles.tile([P, N_CHUNKS * E], F32)
        ones_sb = singles.tile([P, 1], F32)
        nc.vector.memset(ones_sb[:], 1.0)
        stage = singles.tile([P, 2, N_TILES * K], U32)
        w_stage = stage[:, 0, :].bitcast(F32)
        i_stage = stage[:, 1, :]
        # Per-token sum-of-squares (PSUM, written by PE) and rsqrt staging.
        pss_all = ps_single.tile([P, N_TILES], F32)
        ssq_all = singles.tile([P, N_TILES], F32)
        inv_all = singles.tile([P, N_TILES], F32)
        rs_t1 = singles.tile([P, N_TILES], F32)
        rs_t2 = singles.tile([P, N_TILES], F32)

        # rsqrt(ssq) via a quartic in u = ssq/2048. ssq is chi^2(2048), so u
        # lives in [0.87, 1.15] (+-8 sigma of the padded fit range); the
        # Chebyshev fit's max rel err there is 1.7e-6 -- far below both the
        # 2e-4 weight gate and (since top-8 selection uses RAW logits, see
        # phase_b) anything that could flip an index. 5 short DVE ops
        # replace the 11-op Quake+Newton chain that dominated the tail.
        C4, C3, C2, C1, C0 = (
            0.26781764, -1.38896533, 2.93510123, -3.27455575, 2.46060203,
        )

        def inv_tiles_quake(t):
            """Old path: Quake seed + 2 Newton steps (11 DVE ops)."""
            g, gw = t, 1
            nc.vector.tensor_copy(ssq_all[:, g : g + gw], pss_all[:, g : g + gw])
            xs = ssq_all[:, g : g + gw]
            ys = inv_all[:, g : g + gw]
            t1 = rs_t1[:, g : g + gw]
            t2 = rs_t2[:, g : g + gw]
            xu = xs.bitcast(U32)
            yu = ys.bitcast(U32)
            nc.vector.tensor_scalar(
                yu, xu, 1, 0xFFFFFFFF,
                op0=mybir.AluOpType.logical_shift_right,
                op1=mybir.AluOpType.bitwise_xor,
            )
            nc.vector.tensor_scalar(
                yu, yu, 0xFFFFFFFF - 0x5F3759DF, None,
                op0=mybir.AluOpType.subtract,
            )
            for _ in range(NEWTON):
                nc.vector.tensor_mul(t1, xs, ys)
                nc.vector.tensor_mul(t2, t1, ys)
                nc.vector.tensor_scalar(
                    t2, t2, -0.5, 1.5,
                    op0=mybir.AluOpType.mult, op1=mybir.AluOpType.add,
                )
                nc.vector.tensor_mul(ys, ys, t2)

        def inv_tiles(t):
            if INV_MODE == "quake":
                return inv_tiles_quake(t)
            us = small.tile([P, 1], F32, tag="us")
            nc.vector.tensor_scalar(
                us[:], pss_all[:, t : t + 1], 1.0 / 2048.0, None,
                op0=mybir.AluOpType.mult,
            )
            acc = small.tile([P, 1], F32, tag="acc")
            nc.vector.tensor_scalar(
                acc[:], us[:], C4, C3,
                op0=mybir.AluOpType.mult, op1=mybir.AluOpType.add,
            )
            for ck in (C2, C1):
                nc.vector.tensor_scalar(
                    acc[:], acc[:], us[:], ck,
                    op0=mybir.AluOpType.mult, op1=mybir.AluOpType.add,
                )
            nc.vector.tensor_scalar(
                inv_all[:, t : t + 1], acc[:], us[:], C0,
                op0=mybir.AluOpType.mult, op1=mybir.AluOpType.add,
            )

        # DRAM view: ht[(c p), (a t)] -> [p, a, c, t]; one DMA per tile
        # fetches [128p, 16c, 128t] as 512B-contiguous runs.
        hv = ht_d.rearrange("(c p) (a t) -> p a c t", p=P, t=P)
        h_tiles = {}
        sq_tiles = {}

        def phase_a(t):
            """DMA in, logits matmuls, square -> logits PSUM tile."""
            h2 = h_pool.tile([P, N_CHUNKS, P], F32, tag="h_t")
            last = t == N_TILES - 1
            q = N_CHUNKS // 4
            if last:
                # Last tile gates the drain: quarter its DMA and square so
                # each piece is consumed while the stream still runs and the
                # tail only sees the final quarter's chain.
                for j in range(4):
                    nc.sync.dma_start(
                        h2[:, j * q : (j + 1) * q, :],
                        hv[:, t, j * q : (j + 1) * q, :],
                    )
            else:
                nc.sync.dma_start(h2[:, :, :], hv[:, t, :, :])
            h_tiles[t] = h2
            if t == 0:
                # pt rides after the first h tile: it is only needed once
                # tile 0's logits start, and this keeps h streaming first.
                nc.sync.dma_start(pt_sb[:], pt_d[:])
            h_t = h_tiles[t][:, :, :]

            psl = psL_pool.tile([P, E], F32, tag="psl")
            for c in range(N_CHUNKS):
                nc.tensor.matmul(
                    psl[:],
                    lhsT=h_t[:, c, :],
                    rhs=pt_sb[:, c * E : (c + 1) * E],
                    start=(c == 0),
                    stop=(c == N_CHUNKS - 1),
                )

            sq = sq_pool.tile([P, N_CHUNKS, P], F32, tag="sq")
            if last:
                for j in range(4):
                    hq = h_t[:, j * q : (j + 1) * q, :]
                    sqq = sq[:, j * q : (j + 1) * q, :]
                    if j % 2 == 0:
                        nc.scalar.activation(
                            sqq, hq, mybir.ActivationFunctionType.Square
                        )
                    else:
                        nc.vector.tensor_mul(sqq, hq, hq)
            else:
                eng = SQ_PLAN[t % len(SQ_PLAN)]
                if eng == "v":
                    nc.vector.tensor_mul(sq[:, :, :], h_t, h_t)
                elif eng == "g":
                    nc.gpsimd.tensor_mul(sq[:, :, :], h_t, h_t)
                else:
                    nc.scalar.activation(
                        sq[:, :, :], h_t, mybir.ActivationFunctionType.Square
                    )
            sq_tiles[t] = sq
            return psl

        def phase_s(t):
            """ssq[t] = ones.T @ sq chunks, accumulated in PSUM (ap_size=1)."""
            sq = sq_tiles.pop(t)
            for c in range(N_CHUNKS):
                nc.tensor.matmul(
                    pss_all[:, t : t + 1],
                    lhsT=sq[:, c, :],
                    rhs=ones_sb[:],
                    start=(c == 0),
                    stop=(c == N_CHUNKS - 1),
                )

        def phase_b(t, psl):
            """Softmax numerator from PSUM logits, top-8 on the (positive-
            scaled, hence order-identical) probs, scale only the top-8."""
            probs = small.tile([P, E], F32, tag="probs")
            nc.scalar.activation(
                probs[:],
                psl[:],
                mybir.ActivationFunctionType.Exp,
                scale=inv_all[:, t : t + 1],
            )
            den = small.tile([P, 1], F32, tag="den")
            nc.vector.reduce_sum(den[:], probs[:], axis=mybir.AxisListType.X)
            top8 = small.tile([P, K], F32, tag="top8")
            nc.vector.max(out=top8[:], in_=probs[:])
            nc.vector.max_index(
                out=i_stage[:, t * K : (t + 1) * K],
                in_max=top8[:],
                in_values=probs[:],
            )
            rden = small.tile([P, 1], F32, tag="rden")
            nc.vector.reciprocal(rden[:], den[:])
            nc.vector.tensor_scalar_mul(
                w_stage[:, t * K : (t + 1) * K], top8[:], rden[:]
            )

        # Per-tile software pipeline: tile t's ssq/rsqrt/softmax are emitted
        # LAG tiles behind its DMA/logits/square so no engine waits at a
        # tile boundary. Output DMAs ride the ACT queue (SP keeps streaming
        # h uninterrupted); flushed in quarters as their tiles drain.
        def phase_sb(tb, psls):
            phase_s(tb)
            inv_tiles(tb)
            phase_b(tb, psls.pop(tb))
            if tb == N_TILES - 3:
                # Fires on the ACT queue just as the h stream drains -- this
                # transfer hides under the tail's compute instead of
                # stealing a mid-stream DMA slot.
                hi = (N_TILES - 2) * K
                nc.scalar.dma_start(oc_d[:, :, :hi], stage[:, :, :hi])

        for _rep in range(reps):
            psls = {}
            for t in range(N_TILES):
                # tile t-LAG's back phases are emitted BEFORE phase_a(t) so
                # tile t's square never sits ahead of them in an engine queue.
                if t - LAG >= 0:
                    phase_sb(t - LAG, psls)
                psls[t] = phase_a(t)
            for tb in range(N_TILES - LAG, N_TILES):
                phase_sb(tb, psls)

        lo = (N_TILES - 1) * K
        nc.sync.dma_start(oc_d[:, :, lo:], stage[:, :, lo:])

    nc.compile()
    return nc


_CACHE = {}


def _get_program():
    if "nc" not in _CACHE:
        _CACHE["nc"] = build_program()
    return _CACHE["nc"]


def make_inputs_for_cores(hidden_states, proto):
    h = np.asarray(hidden_states, dtype=np.float32)
    p = np.asarray(proto, dtype=np.float32)
    assert h.shape == (T_FULL, D) and p.shape == (E, D)
    norm = np.linalg.norm(p, axis=1, keepdims=True)
    pn = (p / np.maximum(norm, 1e-12)).astype(np.float32)
    # pt[p_, c*64+e] = pn[e, c*128+p_]  -> per-partition rows contiguous
    pt = np.ascontiguousarray(
        pn.T.reshape(N_CHUNKS, P, E).transpose(1, 0, 2)
    ).reshape(P, N_CHUNKS * E)
    return [
        {
            "ht": np.ascontiguousarray(h[c * T_CORE : (c + 1) * T_CORE].T),
            "pt": pt,
        }
        for c in range(N_CORES)
    ]


def unshard_outputs(results):
    w_parts, i_parts = [], []
    for c in range(N_CORES):
        oc = np.asarray(results[c]["out_c"])
        ws = oc[:, 0, :].view(np.float32)
        ix = oc[:, 1, :]
        w_parts.append(ws.reshape(P, N_TILES, K).transpose(1, 0, 2).reshape(T_CORE, K))
        i_parts.append(
            ix.reshape(P, N_TILES, K)
            .transpose(1, 0, 2)
            .reshape(T_CORE, K)
            .astype(np.int32)
        )
    return np.concatenate(w_parts, 0), np.concatenate(i_parts, 0)


def run_on_hw(hidden_states, proto, trace=False):
    from concourse.bass_utils import run_bass_kernel_spmd

    nc = _get_program()
    in_maps = make_inputs_for_cores(hidden_states, proto)
    res = run_bass_kernel_spmd(
        nc, in_maps, core_ids=list(range(N_CORES)), trace=trace
    )
    _CACHE["last_results"] = res
    return unshard_outputs(res.results)


def kernel(hidden_states, proto):
    return run_on_hw(hidden_states, proto, trace=False)


# revision 35
# speedup vs baseline: 1.0534x; 1.0534x over previous
"""CPR router kernel for Trainium2 (8 NeuronCores, data-parallel over tokens).

Math (matches the jax reference):
    h_n = l2norm(hidden_states, axis=1); p_n = l2norm(proto, axis=1)
    logits = h_n @ p_n.T                      # [T, 64] cosine sims
    w = softmax(logits, axis=1)
    routing_weights, selected_experts = top_k(w, 8)

Device strategy (per core, 2048 tokens, 16 tiles of 128 tokens):
    - Host ships h PRE-TRANSPOSED per core: ht[d, t] = h[t, d], so the
      contraction dim d lands on SBUF partitions with no PE transposes and
      no PSUM->SBUF staging copies at all (those were 27us PE + 45us
      DVE/ACT of engine time in the token-major design).
    - proto is tiny: normalized + transposed on host, replicated.
    - Per 128-token tile, one DMA gathers [128p, 16 d-chunks, 128 tok]
      (512B runs -> full DMA efficiency). Then:
        PE: 16 f32 matmuls accumulate logits[128t, 64e] in PSUM
            (lhsT = ht chunk [128d, 128t], rhs = proto chunk [128d, 64e]).
        ACT or DVE (split for balance): sq = ht*ht elementwise.
        PE: 16 matmuls vs a ones[128,1] column reduce sq over partitions
            into ssq[128t, 1] PSUM at ap_size=1 (~free in PE time).
        DVE: rsqrt(ssq) via Quake seed + 2 Newton steps (rel err 4e-6,
            far under the top-k gap scale; no ACT table switch).
        ACT: Exp reads logits straight from PSUM with per-partition
            scale=inv_norm -> softmax numerator.
        DVE: reduce_sum denominator, top-8 max/max_index on the
            (unnormalized) probs -- softmax normalization is a positive
            per-token scalar so the selection is identical -- then scale
            just the top-8 values by 1/den.
    - Outputs stage in ONE u32 buffer [128, 2, 16*8] (w bitcast | idx) so
      each flush is a single DMA; host re-permutes and splits.
    Every engine sits well under the ~49us DMA floor (16.8MB of f32 h at
    358GB/s/core), so the kernel is memory-bound per the target regime.
"""

from contextlib import ExitStack

import numpy as np

import concourse.bass as bass
import concourse.bacc as bacc
import concourse.mybir as mybir
import concourse.tile as tile

N_CORES = 8
T_FULL = 16384
D = 2048
E = 64
K = 8
P = 128
T_CORE = T_FULL // N_CORES  # 2048
N_TILES = T_CORE // P       # 16
N_CHUNKS = D // P           # 16

F32 = mybir.dt.float32
U32 = mybir.dt.uint32

# Engine for each tile's sq = ht*ht pass: "a"=ACT, "v"=DVE, "g"=GPSIMD/Pool.
# ACT also runs the Exps; DVE runs rsqrt + softmax/top-k; Pool is idle.
SQ_PLAN = "avaa" * 4
# Newton steps after the Quake rsqrt seed (2 -> rel err ~4e-6).
NEWTON = 2
# Pipeline lag (tiles) between phase_a (DMA/square/logits) and phase_s
# (ssq matmuls + rsqrt) / phase_b (softmax/top-k).
LAG = 1


INV_MODE = "quake"


def build_program(sq_plan=None, lag=None, newton=None, inv_mode=None, reps=1):
    global SQ_PLAN, LAG, NEWTON, INV_MODE
    if sq_plan is not None:
        SQ_PLAN = sq_plan
    if lag is not None:
        LAG = lag
    if newton is not None:
        NEWTON = newton
    if inv_mode is not None:
        INV_MODE = inv_mode
    nc = bacc.Bacc(
        "TRN2", target_bir_lowering=False, debug=False, num_devices=N_CORES
    )
    ht_d = nc.dram_tensor("ht", [D, T_CORE], F32, kind="ExternalInput").ap()
    pt_d = nc.dram_tensor("pt", [P, N_CHUNKS * E], F32, kind="ExternalInput").ap()
    oc_d = nc.dram_tensor(
        "out_c", [P, 2, N_TILES * K], U32, kind="ExternalOutput"
    ).ap()

    with tile.TileContext(nc) as tc, ExitStack() as ctx:
        singles = ctx.enter_context(tc.tile_pool(name="singles", bufs=1))
        h_pool = ctx.enter_context(tc.tile_pool(name="hin", bufs=5))
        sq_pool = ctx.enter_context(tc.tile_pool(name="sq", bufs=4))
        small = ctx.enter_context(tc.tile_pool(name="small", bufs=4))
        psL_pool = ctx.enter_context(
            tc.tile_pool(name="psL", bufs=6, space=bass.MemorySpace.PSUM)
        )
        ps_single = ctx.enter_context(
            tc.tile_pool(name="pss", bufs=1, space=bass.MemorySpace.PSUM)
        )

        pt_sb = singles.tile([P, N_CHUNKS * E], F32)
        ones_sb = singles.tile([P, 1], F32)
        nc.vector.memset(ones_sb[:], 1.0)
        stage = singles.tile([P, 2, N_TILES * K], U32)
        w_stage = stage[:, 0, :].bitcast(F32)
        i_stage = stage[:, 1, :]
        # Per-token sum-of-squares (PSUM, written by PE) and rsqrt staging.
        pss_all = ps_single.tile([P, N_TILES], F32)
        ssq_all = singles.tile([P, N_TILES], F32)
        inv_all = singles.tile([P, N_TILES], F32)
        rs_t1 = singles.tile([P, N_TILES], F32)
        rs_t2 = singles.tile([P, N_TILES], F32)

        # rsqrt(ssq) via a quartic in u = ssq/2048. ssq is chi^2(2048), so u
        # lives in [0.87, 1.15] (+-8 sigma of the padded fit range); the
        # Chebyshev fit's max rel err there is 1.7e-6 -- far below both the
        # 2e-4 weight gate and (since top-8 selection uses RAW logits, see
        # phase_b) anything that could flip an index. 5 short DVE ops
        # replace the 11-op Quake+Newton chain that dominated the tail.
        C4, C3, C2, C1, C0 = (
            0.26781764, -1.38896533, 2.93510123, -3.27455575, 2.46060203,
        )

        def inv_tiles_quake(t):
            """Old path: Quake seed + 2 Newton steps (11 DVE ops)."""
            g, gw = t, 1
            nc.vector.tensor_copy(ssq_all[:, g : g + gw], pss_all[:, g : g + gw])
            xs = ssq_all[:, g : g + gw]
            ys = inv_all[:, g : g + gw]
            t1 = rs_t1[:, g : g + gw]
            t2 = rs_t2[:, g : g + gw]
            xu = xs.bitcast(U32)
            yu = ys.bitcast(U32)
            nc.vector.tensor_scalar(
                yu, xu, 1, 0xFFFFFFFF,
                op0=mybir.AluOpType.logical_shift_right,
                op1=mybir.AluOpType.bitwise_xor,
            )
            nc.vector.tensor_scalar(
                yu, yu, 0xFFFFFFFF - 0x5F3759DF, None,
                op0=mybir.AluOpType.subtract,
            )
            for _ in range(NEWTON):
                nc.vector.tensor_mul(t1, xs, ys)
                nc.vector.tensor_mul(t2, t1, ys)
                nc.vector.tensor_scalar(
                    t2, t2, -0.5, 1.5,
                    op0=mybir.AluOpType.mult, op1=mybir.AluOpType.add,
                )
                nc.vector.tensor_mul(ys, ys, t2)

        def inv_tiles(t):
            if INV_MODE == "quake":
                return inv_tiles_quake(t)
            us = small.tile([P, 1], F32, tag="us")
            nc.vector.tensor_scalar(
                us[:], pss_all[:, t : t + 1], 1.0 / 2048.0, None,
                op0=mybir.AluOpType.mult,
            )
            acc = small.tile([P, 1], F32, tag="acc")
            nc.vector.tensor_scalar(
                acc[:], us[:], C4, C3,
                op0=mybir.AluOpType.mult, op1=mybir.AluOpType.add,
            )
            for ck in (C2, C1):
                nc.vector.tensor_scalar(
                    acc[:], acc[:], us[:], ck,
                    op0=mybir.AluOpType.mult, op1=mybir.AluOpType.add,
                )
            nc.vector.tensor_scalar(
                inv_all[:, t : t + 1], acc[:], us[:], C0,
                op0=mybir.AluOpType.mult, op1=mybir.AluOpType.add,
            )

        # DRAM view: ht[(c p), (a t)] -> [p, a, c, t]; one DMA per tile
        # fetches [128p, 16c, 128t] as 512B-contiguous runs.
        hv = ht_d.rearrange("(c p) (a t) -> p a c t", p=P, t=P)
        h_tiles = {}
        sq_tiles = {}

        def phase_a(t):
            """DMA in, logits matmuls, square -> logits PSUM tile."""
            h2 = h_pool.tile([P, N_CHUNKS, P], F32, tag="h_t")
            last = t >= N_TILES - 2
            q = N_CHUNKS // 4
            if last:
                # Last tile gates the drain: quarter its DMA and square so
                # each piece is consumed while the stream still runs and the
                # tail only sees the final quarter's chain.
                for j in range(4):
                    nc.sync.dma_start(
                        h2[:, j * q : (j + 1) * q, :],
                        hv[:, t, j * q : (j + 1) * q, :],
                    )
            else:
                nc.sync.dma_start(h2[:, :, :], hv[:, t, :, :])
            h_tiles[t] = h2
            if t == 0:
                # pt rides after the first h tile: it is only needed once
                # tile 0's logits start, and this keeps h streaming first.
                nc.sync.dma_start(pt_sb[:], pt_d[:])
            h_t = h_tiles[t][:, :, :]

            psl = psL_pool.tile([P, E], F32, tag="psl")
            for c in range(N_CHUNKS):
                nc.tensor.matmul(
                    psl[:],
                    lhsT=h_t[:, c, :],
                    rhs=pt_sb[:, c * E : (c + 1) * E],
                    start=(c == 0),
                    stop=(c == N_CHUNKS - 1),
                )

            sq = sq_pool.tile([P, N_CHUNKS, P], F32, tag="sq")
            if last:
                for j in range(4):
                    hq = h_t[:, j * q : (j + 1) * q, :]
                    sqq = sq[:, j * q : (j + 1) * q, :]
                    if j % 2 == 0:
                        nc.scalar.activation(
                            sqq, hq, mybir.ActivationFunctionType.Square
                        )
                    else:
                        nc.vector.tensor_mul(sqq, hq, hq)
            else:
                eng = SQ_PLAN[t % len(SQ_PLAN)]
                if eng == "v":
                    nc.vector.tensor_mul(sq[:, :, :], h_t, h_t)
                elif eng == "g":
                    nc.gpsimd.tensor_mul(sq[:, :, :], h_t, h_t)
                else:
                    nc.scalar.activation(
                        sq[:, :, :], h_t, mybir.ActivationFunctionType.Square
                    )
            sq_tiles[t] = sq
            return psl

        def phase_s(t):
            """ssq[t] = ones.T @ sq chunks, accumulated in PSUM (ap_size=1)."""
            sq = sq_tiles.pop(t)
            for c in range(N_CHUNKS):
                nc.tensor.matmul(
                    pss_all[:, t : t + 1],
                    lhsT=sq[:, c, :],
                    rhs=ones_sb[:],
                    start=(c == 0),
                    stop=(c == N_CHUNKS - 1),
                )

        def phase_b(t, psl):
            """Softmax numerator from PSUM logits, top-8 on the (positive-
            scaled, hence order-identical) probs, scale only the top-8."""
            probs = small.tile([P, E], F32, tag="probs")
            nc.scalar.activation(
                probs[:],
                psl[:],
                mybir.ActivationFunctionType.Exp,
                scale=inv_all[:, t : t + 1],
            )
            den = small.tile([P, 1], F32, tag="den")
            nc.vector.reduce_sum(den[:], probs[:], axis=mybir.AxisListType.X)
            top8 = small.tile([P, K], F32, tag="top8")
            nc.vector.max(out=top8[:], in_=probs[:])
            nc.vector.max_index(
                out=i_stage[:, t * K : (t + 1) * K],
                in_max=top8[:],
                in_values=probs[:],
            )
            rden = small.tile([P, 1], F32, tag="rden")
            nc.vector.reciprocal(rden[:], den[:])
            nc.vector.tensor_scalar_mul(
                w_stage[:, t * K : (t + 1) * K], top8[:], rden[:]
            )

        # Per-tile software pipeline: tile t's ssq/rsqrt/softmax are emitted
        # LAG tiles behind its DMA/logits/square so no engine waits at a
        # tile boundary. Output DMAs ride the ACT queue (SP keeps streaming
        # h uninterrupted); flushed in quarters as their tiles drain.
        def phase_sb(tb, psls):
            phase_s(tb)
            inv_tiles(tb)
            phase_b(tb, psls.pop(tb))
            if tb == N_TILES - 3:
                # Fires on the ACT queue just as the h stream drains -- this
                # transfer hides under the tail's compute instead of
                # stealing a mid-stream DMA slot.
                hi = (N_TILES - 2) * K
                nc.scalar.dma_start(oc_d[:, :, :hi], stage[:, :, :hi])

        for _rep in range(reps):
            psls = {}
            for t in range(N_TILES):
                # tile t-LAG's back phases are emitted BEFORE phase_a(t) so
                # tile t's square never sits ahead of them in an engine queue.
                if t - LAG >= 0:
                    phase_sb(t - LAG, psls)
                psls[t] = phase_a(t)
            for tb in range(N_TILES - LAG, N_TILES):
                phase_sb(tb, psls)

        lo = (N_TILES - 2) * K
        nc.sync.dma_start(oc_d[:, :, lo:], stage[:, :, lo:])

    nc.compile()
    return nc


_CACHE = {}


def _get_program():
    if "nc" not in _CACHE:
        _CACHE["nc"] = build_program()
    return _CACHE["nc"]


def make_inputs_for_cores(hidden_states, proto):
    h = np.asarray(hidden_states, dtype=np.float32)
    p = np.asarray(proto, dtype=np.float32)
    assert h.shape == (T_FULL, D) and p.shape == (E, D)
    norm = np.linalg.norm(p, axis=1, keepdims=True)
    pn = (p / np.maximum(norm, 1e-12)).astype(np.float32)
    # pt[p_, c*64+e] = pn[e, c*128+p_]  -> per-partition rows contiguous
    pt = np.ascontiguousarray(
        pn.T.reshape(N_CHUNKS, P, E).transpose(1, 0, 2)
    ).reshape(P, N_CHUNKS * E)
    return [
        {
            "ht": np.ascontiguousarray(h[c * T_CORE : (c + 1) * T_CORE].T),
            "pt": pt,
        }
        for c in range(N_CORES)
    ]


def unshard_outputs(results):
    w_parts, i_parts = [], []
    for c in range(N_CORES):
        oc = np.asarray(results[c]["out_c"])
        ws = oc[:, 0, :].view(np.float32)
        ix = oc[:, 1, :]
        w_parts.append(ws.reshape(P, N_TILES, K).transpose(1, 0, 2).reshape(T_CORE, K))
        i_parts.append(
            ix.reshape(P, N_TILES, K)
            .transpose(1, 0, 2)
            .reshape(T_CORE, K)
            .astype(np.int32)
        )
    return np.concatenate(w_parts, 0), np.concatenate(i_parts, 0)


def run_on_hw(hidden_states, proto, trace=False):
    from concourse.bass_utils import run_bass_kernel_spmd

    nc = _get_program()
    in_maps = make_inputs_for_cores(hidden_states, proto)
    res = run_bass_kernel_spmd(
        nc, in_maps, core_ids=list(range(N_CORES)), trace=trace
    )
    _CACHE["last_results"] = res
    return unshard_outputs(res.results)


def kernel(hidden_states, proto):
    return run_on_hw(hidden_states, proto, trace=False)
